# revision 86
# baseline (speedup 1.0000x reference)
# Trainium2 Bass kernel for nn_MultiHeadAttentionPure (B=2, S=1024, F=1024, H=16).
#
# The reference splits q/k/v into 64 feature-chunks of 16 ("groups"), runs
# causal attention independently per (group, batch) pair -- 128 independent
# [1024,16] attention problems -- then applies a (buggy-but-faithful) torch
# reshape that scrambles (group, batch, seq) into the [B,S,F] tensor fed to
# the output linear layer.
#
# Sharding (same as previous baseline): core c = b2*4 + qq needs exactly the
# 16 groups {j : j%4 == 2*b2 + qq//2} at input batch b = qq%2 -- a perfect
# partition of the 128 (group, batch) pairs across 8 cores with zero
# cross-core traffic.  Each core computes its 16 attention groups, assembles
# its y^T tile ([1024 features, 256 rows]) on-chip, and runs the output
# linear for its 256 output rows.
#
# Key performance structure (vs the 199us/140us baseline):
#  * Scores matmuls have K=16; two groups run CONCURRENTLY in the PE array
#    via row tiling (tile_position=(0,0) and (64,0)) -> 2x score throughput.
#  * AV matmuls have M=17 (16 v-dims + ones row for the softmax denominator);
#    four groups run concurrently via column tiling (tile_position=(0,32j)),
#    all accumulating into disjoint partition strips of one PSUM bank.
#  * exp() is the true bottleneck (only ACT and DVE can read PSUM).  It is
#    split between ACT (exact exp, free scale=1/A) and DVE (Schraudolph int16
#    bit-trick: scores arrive pre-scaled by A=1024*log2(e) -- folded into q
#    on the host -- so one fused scalar_tensor_tensor computes
#    (psum + B) * tril_mask -> int16, whose bits reinterpreted as fp16 are
#    ~exp(s) with +-3% error; masked entries become exactly +0.0).  A static
#    greedy balancer assigns tiles to ACT/DVE.
#  * Softmax denominators come out of the AV ones-row; they are DMA-gathered,
#    inverted with one reciprocal_approx_fast, and applied on the Pool engine
#    (which cannot touch PSUM) from DMA'd SBUF copies using 0-stride
#    partition-broadcast reads.
#  * Output linear: bias is added via a K=1 matmul (ones x b_out) and the
#    PSUM result is DMA'd straight to DRAM.
import numpy as np

B, S, F, H = 2, 1024, 1024, 16
NG = 16          # groups per core
P = 128
NCORES = 8

A_SCALE = 1024.0 * np.log2(np.e)           # 1477.3197
# Schraudolph bias: 15*1024 (fp16 exponent bias<<10) - geometric centering
# of the (1+f)/2^f error sawtooth + 0.5 so truncation acts as rounding.
B_SCH = 15360.0 - 1024.0 * np.log2(np.exp(0.02985)) + 0.5


def _core_groups(c):
    b2, qq = c // 4, c % 4
    b = qq % 2
    jmod = 2 * b2 + qq // 2
    js = [4 * h2 + jmod for h2 in range(NG)]
    return b2, qq, b, js


def _exp_plan(causal):
    """Exp-pass engine plan: one op per score tile.  Diagonal tiles must go
    to DVE (Schraudolph with fused causal mask); off-diagonal tiles are
    greedily balanced between ACT (exact exp) and DVE (Schraudolph).
    Returns {(c, q, t, ti): 'A' | 'D'}."""
    busy = {'A': 4600.0, 'D': 5300.0}   # av-copies on ACT; recips on DVE
    rate = {'A': 1.2, 'D': 0.96}
    ovh = {'A': 352.0, 'D': 140.0}
    plan = {}
    for c in range(2):
        for q in range(4):
            ntile = 4 * c + 4 if causal else 8
            for t in range(ntile):
                d = t - 4 * c
                a1 = 128 * d if (causal and d >= 0) else 0
                diag = causal and d >= 0
                size = 2 * (512 - a1)
                for ti in range(2):
                    if diag:
                        eng = 'D'
                    else:
                        eng = min('AD', key=lambda e: busy[e] + (size + ovh[e]) / rate[e])
                    busy[eng] += (size + ovh[eng]) / rate[eng]
                    plan[(c, q, t, ti)] = eng
    return plan


def _build(causal: bool, n_iter: int = 1):
    import concourse.bass as bass
    import concourse.mybir as mybir
    from concourse import bacc, tile

    F32 = mybir.dt.float32
    F16 = mybir.dt.float16
    I16 = mybir.dt.int16
    AF = mybir.ActivationFunctionType
    ADD = mybir.AluOpType.add
    MUL = mybir.AluOpType.mult

    nc = bacc.Bacc("TRN2", target_bir_lowering=False, debug=False)
    qt = nc.declare_dram_parameter("qt", [P, 8 * S], F16, isOutput=False)
    kt = nc.declare_dram_parameter("kt", [P, 8 * S], F16, isOutput=False)
    va = nc.declare_dram_parameter("va", [P, 8 * NG * 32], F16, isOutput=False)
    wt = nc.declare_dram_parameter("wt", [P, 16 * F], F16, isOutput=False)
    msk = nc.declare_dram_parameter("msk", [P, 1024], F16, isOutput=False)
    bv = nc.declare_dram_parameter("bv", [1, F], F16, isOutput=False)
    on1 = nc.declare_dram_parameter("on1", [1, P], F16, isOutput=False)
    seld = nc.declare_dram_parameter("seld", [P, P], F16, isOutput=False)
    out = nc.declare_dram_parameter("o", [256, F], F32, isOutput=True)

    plan = _exp_plan(causal)

    import contextlib
    with tile.TileContext(nc) as tc:
        loop_ctx = tc.For_i(0, n_iter, 1, hint_engines=(
            mybir.EngineType.PE, mybir.EngineType.DVE, mybir.EngineType.Activation,
            mybir.EngineType.SP, mybir.EngineType.Pool,
        )) if n_iter > 1 else contextlib.nullcontext()
        with loop_ctx, \
             tc.tile_pool(name="cst", bufs=1) as cst, \
             tc.tile_pool(name="expp", bufs=6) as expp, \
             tc.tile_pool(name="wk", bufs=3) as wkp, \
             tc.tile_pool(name="ytp", bufs=1) as ytp, \
             tc.tile_pool(name="scps", bufs=2, space="PSUM") as scps, \
             tc.tile_pool(name="avps", bufs=2, space="PSUM") as avps, \
             tc.tile_pool(name="auxps", bufs=2, space="PSUM") as auxps:

            qt_sb = cst.tile([P, 8, S], F16)
            kt_sb = cst.tile([P, 8, S], F16)
            va_sb = cst.tile([P, 8, NG, 32], F16)
            wt_sb = cst.tile([P, 16, F], F16)
            msk_sb = cst.tile([P, 2, 512], F16)
            bv_sb = cst.tile([1, F], F16)
            on1_sb = cst.tile([1, P], F16)
            seld_sb = cst.tile([P, P], F16)
            nc.sync.dma_start(seld_sb[:], seld[:])
            nc.sync.dma_start(msk_sb[:], msk.rearrange("p (a b) -> p a b", a=2))
            # q/k full-width per-quad chunks so compute starts early
            for qq in range(4):
                nc.sync.dma_start(
                    qt_sb[:, 2 * qq:2 * qq + 2, :],
                    qt[:, 2 * qq * S:(2 * qq + 2) * S].rearrange(
                        "p (a b) -> p a b", a=2))
                nc.sync.dma_start(
                    kt_sb[:, 2 * qq:2 * qq + 2, :],
                    kt[:, 2 * qq * S:(2 * qq + 2) * S].rearrange(
                        "p (a b) -> p a b", a=2))
            nc.scalar.dma_start(va_sb[:],
                                va.rearrange("p (a g h) -> p a g h", a=8, g=NG))
            nc.scalar.dma_start(bv_sb[:], bv[:])
            nc.scalar.dma_start(on1_sb[:], on1[:])
            # wt is only needed by the output linear -- load it last
            nc.scalar.dma_start(wt_sb[:], wt.rearrange("p (a b) -> p a b", a=16))

            # doubled y^T: 16 blocks of 128 rows; rows 32m+16..32m+31 of each
            # block are denom*recip junk whose wt rows are zero
            yt_sb = ytp.tile([P, 16, 256], F16)

            for c in range(2):
                ntile = 4 * c + 4 if causal else 8
                for q in range(4):
                    av_ps = avps.tile([P, 512], F32, tag="av")
                    expts = {}
                    for t in range(ntile):
                        d = t - 4 * c
                        a1 = 128 * d if (causal and d >= 0) else 0
                        diag = causal and d >= 0
                        for ti in range(2):
                            pr = 2 * q + ti
                            sc = scps.tile([P, 2, 512], F32, tag="sc")
                            for u in range(2):
                                nc.tensor.matmul(
                                    sc[:, u, a1:],
                                    kt_sb[64 * u:64 * u + 16, pr, t * P:(t + 1) * P],
                                    qt_sb[64 * u:64 * u + 16, pr,
                                          512 * c + a1: 512 * (c + 1)],
                                    start=True, stop=True,
                                    tile_position=(64 * u, 0))
                            ex = expp.tile([P, 2, 512], F16, tag="expt")
                            eng = plan[(c, q, t, ti)]
                            if eng == 'A':
                                nc.scalar.activation(
                                    ex[:, :, a1:], sc[:, :, a1:], AF.Exp,
                                    scale=float(1.0 / A_SCALE))
                            elif diag:
                                nc.vector.scalar_tensor_tensor(
                                    out=ex[:, :, a1:].bitcast(I16),
                                    in0=sc[:, :, a1:], scalar=float(B_SCH),
                                    in1=msk_sb[:, :, 0:512 - a1],
                                    op0=ADD, op1=MUL)
                            else:
                                nc.vector.tensor_scalar(
                                    out=ex[:, :, a1:].bitcast(I16),
                                    in0=sc[:, :, a1:], scalar1=float(B_SCH),
                                    scalar2=None, op0=ADD)
                            expts[(t, ti)] = ex
                        for j in range(4):
                            exj = expts[(t, j // 2)]
                            nc.tensor.matmul(
                                av_ps[32 * j:32 * j + 32, a1:],
                                va_sb[:, t, 4 * q + j, :],
                                exj[:, j % 2, a1:],
                                start=(t == 0), stop=(t == ntile - 1),
                                skip_group_check=True,
                                tile_position=(0, 32 * j))
                    # denominators: gather-matmul broadcasts each group's
                    # denom row to all 32 rows of its strip, then one
                    # reciprocal; normalization writes straight into yt
                    av_sbh = wkp.tile([P, 512], F16, tag="avsb")
                    nc.scalar.copy(av_sbh[:], av_ps[:])
                    db_ps = auxps.tile([P, 512], F32, tag="aux")
                    nc.tensor.matmul(db_ps[:], seld_sb[:], av_sbh[:],
                                     start=True, stop=True)
                    rb_sb = wkp.tile([P, 512], F32, tag="rb")
                    nc.vector.reciprocal_approx_fast(out=rb_sb[:], in_=db_ps[:])
                    for j in range(4):
                        i = 4 * q + j
                        for m in range(4):
                            nc.gpsimd.tensor_tensor(
                                out=yt_sb[32 * m:32 * m + 32, i,
                                          128 * c:128 * (c + 1)],
                                in0=av_sbh[32 * j:32 * j + 32, m:512:4],
                                in1=rb_sb[32 * j:32 * j + 32, m:512:4],
                                op=MUL)
                # output linear for rows 128c..128c+127 (yt cols of this c)
                for oc in range(2):
                    lp = auxps.tile([P, 512], F32, tag="aux")
                    for ft in range(16):
                        nc.tensor.matmul(
                            lp[:], yt_sb[:, ft, c * P:(c + 1) * P],
                            wt_sb[:, ft, oc * 512:(oc + 1) * 512],
                            start=(ft == 0), stop=False)
                    nc.tensor.matmul(
                        lp[:], on1_sb[:], bv_sb[0:1, oc * 512:(oc + 1) * 512],
                        start=False, stop=True)
                    lo_sb = wkp.tile([P, 512], F32, tag="lo")
                    nc.scalar.copy(lo_sb[:], lp[:])
                    nc.sync.dma_start(
                        out[c * P:(c + 1) * P, oc * 512:(oc + 1) * 512], lo_sb[:])
    nc.compile()
    return nc


_NC_CACHE = {}


def _get_nc(causal: bool, n_iter: int = 1):
    key = (causal, n_iter)
    if key not in _NC_CACHE:
        _NC_CACHE[key] = _build(causal, n_iter)
    return _NC_CACHE[key]


def _shard_inputs(q, k, v, W_out, b_out):
    """Build the 8 per-core input maps."""
    # wt blocks match yt: block i (local group), row 32m+h = feature
    # 128*(i//2) + 64*(i%2) + 16m + h; rows 32m+16..31 are zero (junk rows)
    wtf = np.ascontiguousarray(W_out.T).astype(np.float16)   # wt[f, o]
    wtp = np.zeros((P, 16 * F), np.float16)
    for i in range(NG):
        for m in range(4):
            f0 = 128 * (i // 2) + 64 * (i % 2) + 16 * m
            wtp[32 * m:32 * m + 16, i * F:(i + 1) * F] = wtf[f0:f0 + 16, :]
    ri, ci = np.mgrid[0:P, 0:512]
    mhalf = np.where((ci >= 128) | (ci >= ri), 1.0, 0.0).astype(np.float16)
    mskv = np.concatenate([mhalf, mhalf], axis=1)            # [128, 1024]
    bvv = b_out.astype(np.float16).reshape(1, F)
    onv = np.ones((1, P), np.float16)
    seldv = np.zeros((P, P), np.float16)
    for p in range(P):
        seldv[32 * (p // 32) + 16, p] = 1.0

    in_maps = []
    for c in range(NCORES):
        _, _, b, js = _core_groups(c)
        cols = np.concatenate([j * H + np.arange(H) for j in js])
        qc = q[b][:, cols]        # [S, 256] feature order: local group i, h
        kc = k[b][:, cols]
        vc = v[b][:, cols]
        qtp = np.zeros((P, 8 * S), np.float16)
        ktp = np.zeros((P, 8 * S), np.float16)
        for i in range(NG):
            pr, u = i // 2, i % 2
            qtp[64 * u:64 * u + 16, pr * S:(pr + 1) * S] = \
                (qc[:, 16 * i:16 * (i + 1)].T * (A_SCALE / 4.0)).astype(np.float16)
            ktp[64 * u:64 * u + 16, pr * S:(pr + 1) * S] = \
                kc[:, 16 * i:16 * (i + 1)].T.astype(np.float16)
        vap = np.zeros((P, 8, NG, 32), np.float16)
        for t in range(8):
            blk = vc[t * P:(t + 1) * P, :].reshape(P, NG, H)
            vap[:, t, :, :16] = blk.astype(np.float16)
            vap[:, t, :, 16] = 1.0
        in_maps.append({
            "qt": qtp,
            "kt": ktp,
            "va": vap.reshape(P, 8 * NG * 32),
            "wt": wtp,
            "msk": mskv,
            "bv": bvv,
            "on1": onv,
            "seld": seldv,
        })
    return in_maps


def _unshard(outs):
    full = np.empty((B, S, F), np.float32)
    for c in range(NCORES):
        b2, qq, _, _ = _core_groups(c)
        full[b2, 256 * qq:256 * (qq + 1), :] = outs[c]
    return full


def _numpy_core(in_map, causal=True):
    """Numpy emulation of the device program (host-logic validation)."""
    qtp = in_map["qt"].astype(np.float32).reshape(P, 8, S)
    ktp = in_map["kt"].astype(np.float32).reshape(P, 8, S)
    vap = in_map["va"].astype(np.float32).reshape(P, 8, NG, 32)
    wtp = in_map["wt"].astype(np.float32).reshape(P, 16, F)
    mskv = in_map["msk"].astype(np.float32).reshape(P, 2, 512)
    bvv = in_map["bv"].astype(np.float32)[0]
    plan = _exp_plan(causal)
    ytv = np.zeros((P, 16, 256), np.float32)
    for c in range(2):
        ntile = 4 * c + 4 if causal else 8
        for q in range(4):
            av = np.zeros((P, 512), np.float32)
            for t in range(ntile):
                d = t - 4 * c
                a1 = 128 * d if (causal and d >= 0) else 0
                diag = causal and d >= 0
                exts = {}
                for ti in range(2):
                    pr = 2 * q + ti
                    sc = np.zeros((P, 2, 512), np.float32)
                    for u in range(2):
                        sc[:, u, a1:] = (
                            ktp[64 * u:64 * u + 16, pr, t * P:(t + 1) * P].T
                            @ qtp[64 * u:64 * u + 16, pr, 512 * c + a1:512 * (c + 1)])
                    ex = np.zeros((P, 2, 512), np.float32)
                    eng = plan[(c, q, t, ti)]
                    if eng == 'A':
                        ex[:, :, a1:] = np.exp(sc[:, :, a1:] / A_SCALE).astype(
                            np.float16).astype(np.float32)
                    else:
                        tt = sc[:, :, a1:] + B_SCH
                        if diag:
                            tt = tt * mskv[:, :, 0:512 - a1]
                        ex[:, :, a1:] = np.floor(tt).astype(np.int16).view(
                            np.float16).astype(np.float32)
                    exts[ti] = ex
                for j in range(4):
                    av[32 * j:32 * j + 32, a1:] += (
                        vap[:, t, 4 * q + j, :].T @ exts[j // 2][:, j % 2, a1:])
            avh = av.astype(np.float16).astype(np.float32)
            for j in range(4):
                recip = (1.0 / avh[32 * j + 16, :])
                xsj = (avh[32 * j:32 * j + 32, :] * recip[None, :]).astype(np.float16)
                i = 4 * q + j
                for m in range(4):
                    ytv[32 * m:32 * m + 32, i, 128 * c:128 * (c + 1)] = \
                        xsj[:, m:512:4].astype(np.float32)
    o = np.zeros((256, F), np.float32)
    for c in range(2):
        for oc in range(2):
            acc = np.zeros((P, 512), np.float32)
            for ft in range(16):
                acc += (ytv[:, ft, c * P:(c + 1) * P]
                        .astype(np.float16).astype(np.float32).T
                        @ wtp[:, ft, oc * 512:(oc + 1) * 512])
            o[c * P:(c + 1) * P, oc * 512:(oc + 1) * 512] = acc + bvv[oc * 512:(oc + 1) * 512]
    return o


def kernel(q, k, v, W_out, b_out, apply_mask, _mock=False):
    q = np.asarray(q, np.float32)
    k = np.asarray(k, np.float32)
    v = np.asarray(v, np.float32)
    W_out = np.asarray(W_out, np.float32)
    b_out = np.asarray(b_out, np.float32)
    causal = bool(int(np.asarray(apply_mask)))
    in_maps = _shard_inputs(q, k, v, W_out, b_out)
    if _mock:
        outs = [_numpy_core(m, causal) for m in in_maps]
        return _unshard(outs)
    from concourse.bass_utils import run_bass_kernel_spmd
    nc = _get_nc(causal)
    res = run_bass_kernel_spmd(nc, in_maps, core_ids=list(range(NCORES)))
    return _unshard([r["o"] for r in res.results])


# revision 88
# speedup vs baseline: 1.0985x; 1.0985x over previous
# Trainium2 Bass kernel for nn_MultiHeadAttentionPure (B=2, S=1024, F=1024, H=16).
#
# The reference splits q/k/v into 64 feature-chunks of 16 ("groups"), runs
# causal attention independently per (group, batch) pair -- 128 independent
# [1024,16] attention problems -- then applies a (buggy-but-faithful) torch
# reshape that scrambles (group, batch, seq) into the [B,S,F] tensor fed to
# the output linear layer.
#
# Sharding (same as previous baseline): core c = b2*4 + qq needs exactly the
# 16 groups {j : j%4 == 2*b2 + qq//2} at input batch b = qq%2 -- a perfect
# partition of the 128 (group, batch) pairs across 8 cores with zero
# cross-core traffic.  Each core computes its 16 attention groups, assembles
# its y^T tile ([1024 features, 256 rows]) on-chip, and runs the output
# linear for its 256 output rows.
#
# Key performance structure (vs the 199us/140us baseline):
#  * Scores matmuls have K=16; two groups run CONCURRENTLY in the PE array
#    via row tiling (tile_position=(0,0) and (64,0)) -> 2x score throughput.
#  * AV matmuls have M=17 (16 v-dims + ones row for the softmax denominator);
#    four groups run concurrently via column tiling (tile_position=(0,32j)),
#    all accumulating into disjoint partition strips of one PSUM bank.
#  * exp() is the true bottleneck (only ACT and DVE can read PSUM).  It is
#    split between ACT (exact exp, free scale=1/A) and DVE (Schraudolph int16
#    bit-trick: scores arrive pre-scaled by A=1024*log2(e) -- folded into q
#    on the host -- so one fused scalar_tensor_tensor computes
#    (psum + B) * tril_mask -> int16, whose bits reinterpreted as fp16 are
#    ~exp(s) with +-3% error; masked entries become exactly +0.0).  A static
#    greedy balancer assigns tiles to ACT/DVE.
#  * Softmax denominators come out of the AV ones-row; they are DMA-gathered,
#    inverted with one reciprocal_approx_fast, and applied on the Pool engine
#    (which cannot touch PSUM) from DMA'd SBUF copies using 0-stride
#    partition-broadcast reads.
#  * Output linear: bias is added via a K=1 matmul (ones x b_out) and the
#    PSUM result is DMA'd straight to DRAM.
import numpy as np

B, S, F, H = 2, 1024, 1024, 16
NG = 16          # groups per core
P = 128
NCORES = 8

A_SCALE = 1024.0 * np.log2(np.e)           # 1477.3197
# Schraudolph bias: 15*1024 (fp16 exponent bias<<10) - geometric centering
# of the (1+f)/2^f error sawtooth + 0.5 so truncation acts as rounding.
B_SCH = 15360.0 - 1024.0 * np.log2(np.exp(0.02985)) + 0.5


def _core_groups(c):
    b2, qq = c // 4, c % 4
    b = qq % 2
    jmod = 2 * b2 + qq // 2
    js = [4 * h2 + jmod for h2 in range(NG)]
    return b2, qq, b, js


def _exp_plan(causal):
    """Exp-pass engine plan: one op per score tile.  Diagonal tiles must go
    to DVE (Schraudolph with fused causal mask); off-diagonal tiles are
    greedily balanced between ACT (exact exp) and DVE (Schraudolph).
    Returns {(c, q, t, ti): 'A' | 'D'}."""
    busy = {'A': 4600.0, 'D': 5300.0}   # av-copies on ACT; recips on DVE
    rate = {'A': 1.2, 'D': 0.96}
    ovh = {'A': 352.0, 'D': 140.0}
    plan = {}
    for c in range(2):
        for q in range(4):
            ntile = 4 * c + 4 if causal else 8
            for t in range(ntile):
                d = t - 4 * c
                a1 = 128 * d if (causal and d >= 0) else 0
                diag = causal and d >= 0
                size = 2 * (512 - a1)
                for ti in range(2):
                    if diag:
                        eng = 'D'
                    else:
                        eng = min('AD', key=lambda e: busy[e] + (size + ovh[e]) / rate[e])
                    busy[eng] += (size + ovh[eng]) / rate[eng]
                    plan[(c, q, t, ti)] = eng
    return plan


def _build(causal: bool, n_iter: int = 1):
    import concourse.bass as bass
    import concourse.mybir as mybir
    from concourse import bacc, tile

    F32 = mybir.dt.float32
    F16 = mybir.dt.float16
    I16 = mybir.dt.int16
    AF = mybir.ActivationFunctionType
    ADD = mybir.AluOpType.add
    MUL = mybir.AluOpType.mult

    nc = bacc.Bacc("TRN2", target_bir_lowering=False, debug=False)
    qt = nc.declare_dram_parameter("qt", [P, 8 * S], F16, isOutput=False)
    kt = nc.declare_dram_parameter("kt", [P, 8 * S], F16, isOutput=False)
    va = nc.declare_dram_parameter("va", [P, 8 * NG * 32], F16, isOutput=False)
    wt = nc.declare_dram_parameter("wt", [P, 16 * F], F16, isOutput=False)
    msk = nc.declare_dram_parameter("msk", [P, 1024], F16, isOutput=False)
    bv = nc.declare_dram_parameter("bv", [1, F], F16, isOutput=False)
    on1 = nc.declare_dram_parameter("on1", [1, P], F16, isOutput=False)
    seld = nc.declare_dram_parameter("seld", [P, P], F16, isOutput=False)
    out = nc.declare_dram_parameter("o", [256, F], F32, isOutput=True)

    plan = _exp_plan(causal)

    import contextlib
    with tile.TileContext(nc) as tc:
        loop_ctx = tc.For_i(0, n_iter, 1, hint_engines=(
            mybir.EngineType.PE, mybir.EngineType.DVE, mybir.EngineType.Activation,
            mybir.EngineType.SP, mybir.EngineType.Pool,
        )) if n_iter > 1 else contextlib.nullcontext()
        with loop_ctx, \
             tc.tile_pool(name="cst", bufs=1) as cst, \
             tc.tile_pool(name="expp", bufs=6) as expp, \
             tc.tile_pool(name="wk", bufs=3) as wkp, \
             tc.tile_pool(name="ytp", bufs=1) as ytp, \
             tc.tile_pool(name="scps", bufs=3, space="PSUM") as scps, \
             tc.tile_pool(name="avps", bufs=1, space="PSUM") as avps, \
             tc.tile_pool(name="auxps", bufs=1, space="PSUM") as auxps:

            qt_sb = cst.tile([P, 8, S], F16)
            kt_sb = cst.tile([P, 8, S], F16)
            va_sb = cst.tile([P, 8, NG, 32], F16)
            wt_sb = cst.tile([P, 16, F], F16)
            msk_sb = cst.tile([P, 2, 512], F16)
            bv_sb = cst.tile([1, F], F16)
            on1_sb = cst.tile([1, P], F16)
            seld_sb = cst.tile([P, P], F16)
            nc.sync.dma_start(seld_sb[:], seld[:])
            nc.sync.dma_start(msk_sb[:], msk.rearrange("p (a b) -> p a b", a=2))
            # q/k full-width per-quad chunks so compute starts early
            for qq in range(4):
                nc.sync.dma_start(
                    qt_sb[:, 2 * qq:2 * qq + 2, :],
                    qt[:, 2 * qq * S:(2 * qq + 2) * S].rearrange(
                        "p (a b) -> p a b", a=2))
                nc.sync.dma_start(
                    kt_sb[:, 2 * qq:2 * qq + 2, :],
                    kt[:, 2 * qq * S:(2 * qq + 2) * S].rearrange(
                        "p (a b) -> p a b", a=2))
            nc.scalar.dma_start(va_sb[:],
                                va.rearrange("p (a g h) -> p a g h", a=8, g=NG))
            nc.scalar.dma_start(bv_sb[:], bv[:])
            nc.scalar.dma_start(on1_sb[:], on1[:])
            # wt is only needed by the output linear -- load it last
            nc.scalar.dma_start(wt_sb[:], wt.rearrange("p (a b) -> p a b", a=16))

            # doubled y^T: 16 blocks of 128 rows; rows 32m+16..32m+31 of each
            # block are denom*recip junk whose wt rows are zero
            yt_sb = ytp.tile([P, 16, 256], F16)

            for c in (1, 0):
                ntile = 4 * c + 4 if causal else 8
                for q in range(4):
                    av_ps = avps.tile([P, 512], F32, tag="av")
                    expts = {}
                    for t in range(ntile):
                        d = t - 4 * c
                        a1 = 128 * d if (causal and d >= 0) else 0
                        diag = causal and d >= 0
                        for ti in range(2):
                            pr = 2 * q + ti
                            sc = scps.tile([P, 2, 512], F32, tag="sc")
                            for u in range(2):
                                nc.tensor.matmul(
                                    sc[:, u, a1:],
                                    kt_sb[64 * u:64 * u + 16, pr, t * P:(t + 1) * P],
                                    qt_sb[64 * u:64 * u + 16, pr,
                                          512 * c + a1: 512 * (c + 1)],
                                    start=True, stop=True,
                                    tile_position=(64 * u, 0))
                            ex = expp.tile([P, 2, 512], F16, tag="expt")
                            eng = plan[(c, q, t, ti)]
                            if eng == 'A':
                                nc.scalar.activation(
                                    ex[:, :, a1:], sc[:, :, a1:], AF.Exp,
                                    scale=float(1.0 / A_SCALE))
                            elif diag:
                                nc.vector.scalar_tensor_tensor(
                                    out=ex[:, :, a1:].bitcast(I16),
                                    in0=sc[:, :, a1:], scalar=float(B_SCH),
                                    in1=msk_sb[:, :, 0:512 - a1],
                                    op0=ADD, op1=MUL)
                            else:
                                nc.vector.tensor_scalar(
                                    out=ex[:, :, a1:].bitcast(I16),
                                    in0=sc[:, :, a1:], scalar1=float(B_SCH),
                                    scalar2=None, op0=ADD)
                            expts[(t, ti)] = ex
                        for j in range(4):
                            exj = expts[(t, j // 2)]
                            nc.tensor.matmul(
                                av_ps[32 * j:32 * j + 32, a1:],
                                va_sb[:, t, 4 * q + j, :],
                                exj[:, j % 2, a1:],
                                start=(t == 0), stop=(t == ntile - 1),
                                skip_group_check=True,
                                tile_position=(0, 32 * j))
                    # denominators: gather-matmul broadcasts each group's
                    # denom row to all 32 rows of its strip, then one
                    # reciprocal; normalization writes straight into yt
                    av_sbh = wkp.tile([P, 512], F16, tag="avsb")
                    nc.scalar.copy(av_sbh[:], av_ps[:])
                    db_ps = auxps.tile([P, 512], F32, tag="aux")
                    nc.tensor.matmul(db_ps[:], seld_sb[:], av_sbh[:],
                                     start=True, stop=True)
                    rb_sb = wkp.tile([P, 512], F32, tag="rb")
                    nc.vector.reciprocal_approx_fast(out=rb_sb[:], in_=db_ps[:])
                    for j in range(4):
                        i = 4 * q + j
                        for m in range(4):
                            nc.gpsimd.tensor_tensor(
                                out=yt_sb[32 * m:32 * m + 32, i,
                                          128 * c:128 * (c + 1)],
                                in0=av_sbh[32 * j:32 * j + 32, m:512:4],
                                in1=rb_sb[32 * j:32 * j + 32, m:512:4],
                                op=MUL)
                # output linear for rows 128c..128c+127 (yt cols of this c)
                for oc in range(2):
                    lp = auxps.tile([P, 512], F32, tag="aux")
                    for ft in range(16):
                        nc.tensor.matmul(
                            lp[:], yt_sb[:, ft, c * P:(c + 1) * P],
                            wt_sb[:, ft, oc * 512:(oc + 1) * 512],
                            start=(ft == 0), stop=False)
                    nc.tensor.matmul(
                        lp[:], on1_sb[:], bv_sb[0:1, oc * 512:(oc + 1) * 512],
                        start=False, stop=True)
                    lo_sb = wkp.tile([P, 512], F32, tag="lo")
                    nc.scalar.copy(lo_sb[:], lp[:])
                    nc.sync.dma_start(
                        out[c * P:(c + 1) * P, oc * 512:(oc + 1) * 512], lo_sb[:])
    nc.compile()
    return nc


_NC_CACHE = {}


def _get_nc(causal: bool, n_iter: int = 1):
    key = (causal, n_iter)
    if key not in _NC_CACHE:
        _NC_CACHE[key] = _build(causal, n_iter)
    return _NC_CACHE[key]


def _shard_inputs(q, k, v, W_out, b_out):
    """Build the 8 per-core input maps."""
    # wt blocks match yt: block i (local group), row 32m+h = feature
    # 128*(i//2) + 64*(i%2) + 16m + h; rows 32m+16..31 are zero (junk rows)
    wtf = np.ascontiguousarray(W_out.T).astype(np.float16)   # wt[f, o]
    wtp = np.zeros((P, 16 * F), np.float16)
    for i in range(NG):
        for m in range(4):
            f0 = 128 * (i // 2) + 64 * (i % 2) + 16 * m
            wtp[32 * m:32 * m + 16, i * F:(i + 1) * F] = wtf[f0:f0 + 16, :]
    ri, ci = np.mgrid[0:P, 0:512]
    mhalf = np.where((ci >= 128) | (ci >= ri), 1.0, 0.0).astype(np.float16)
    mskv = np.concatenate([mhalf, mhalf], axis=1)            # [128, 1024]
    bvv = b_out.astype(np.float16).reshape(1, F)
    onv = np.ones((1, P), np.float16)
    seldv = np.zeros((P, P), np.float16)
    for p in range(P):
        seldv[32 * (p // 32) + 16, p] = 1.0

    in_maps = []
    for c in range(NCORES):
        _, _, b, js = _core_groups(c)
        cols = np.concatenate([j * H + np.arange(H) for j in js])
        qc = q[b][:, cols]        # [S, 256] feature order: local group i, h
        kc = k[b][:, cols]
        vc = v[b][:, cols]
        qtp = np.zeros((P, 8 * S), np.float16)
        ktp = np.zeros((P, 8 * S), np.float16)
        for i in range(NG):
            pr, u = i // 2, i % 2
            qtp[64 * u:64 * u + 16, pr * S:(pr + 1) * S] = \
                (qc[:, 16 * i:16 * (i + 1)].T * (A_SCALE / 4.0)).astype(np.float16)
            ktp[64 * u:64 * u + 16, pr * S:(pr + 1) * S] = \
                kc[:, 16 * i:16 * (i + 1)].T.astype(np.float16)
        vap = np.zeros((P, 8, NG, 32), np.float16)
        for t in range(8):
            blk = vc[t * P:(t + 1) * P, :].reshape(P, NG, H)
            vap[:, t, :, :16] = blk.astype(np.float16)
            vap[:, t, :, 16] = 1.0
        in_maps.append({
            "qt": qtp,
            "kt": ktp,
            "va": vap.reshape(P, 8 * NG * 32),
            "wt": wtp,
            "msk": mskv,
            "bv": bvv,
            "on1": onv,
            "seld": seldv,
        })
    return in_maps


def _unshard(outs):
    full = np.empty((B, S, F), np.float32)
    for c in range(NCORES):
        b2, qq, _, _ = _core_groups(c)
        full[b2, 256 * qq:256 * (qq + 1), :] = outs[c]
    return full


def _numpy_core(in_map, causal=True):
    """Numpy emulation of the device program (host-logic validation)."""
    qtp = in_map["qt"].astype(np.float32).reshape(P, 8, S)
    ktp = in_map["kt"].astype(np.float32).reshape(P, 8, S)
    vap = in_map["va"].astype(np.float32).reshape(P, 8, NG, 32)
    wtp = in_map["wt"].astype(np.float32).reshape(P, 16, F)
    mskv = in_map["msk"].astype(np.float32).reshape(P, 2, 512)
    bvv = in_map["bv"].astype(np.float32)[0]
    plan = _exp_plan(causal)
    ytv = np.zeros((P, 16, 256), np.float32)
    for c in range(2):
        ntile = 4 * c + 4 if causal else 8
        for q in range(4):
            av = np.zeros((P, 512), np.float32)
            for t in range(ntile):
                d = t - 4 * c
                a1 = 128 * d if (causal and d >= 0) else 0
                diag = causal and d >= 0
                exts = {}
                for ti in range(2):
                    pr = 2 * q + ti
                    sc = np.zeros((P, 2, 512), np.float32)
                    for u in range(2):
                        sc[:, u, a1:] = (
                            ktp[64 * u:64 * u + 16, pr, t * P:(t + 1) * P].T
                            @ qtp[64 * u:64 * u + 16, pr, 512 * c + a1:512 * (c + 1)])
                    ex = np.zeros((P, 2, 512), np.float32)
                    eng = plan[(c, q, t, ti)]
                    if eng == 'A':
                        ex[:, :, a1:] = np.exp(sc[:, :, a1:] / A_SCALE).astype(
                            np.float16).astype(np.float32)
                    else:
                        tt = sc[:, :, a1:] + B_SCH
                        if diag:
                            tt = tt * mskv[:, :, 0:512 - a1]
                        ex[:, :, a1:] = np.floor(tt).astype(np.int16).view(
                            np.float16).astype(np.float32)
                    exts[ti] = ex
                for j in range(4):
                    av[32 * j:32 * j + 32, a1:] += (
                        vap[:, t, 4 * q + j, :].T @ exts[j // 2][:, j % 2, a1:])
            avh = av.astype(np.float16).astype(np.float32)
            for j in range(4):
                recip = (1.0 / avh[32 * j + 16, :])
                xsj = (avh[32 * j:32 * j + 32, :] * recip[None, :]).astype(np.float16)
                i = 4 * q + j
                for m in range(4):
                    ytv[32 * m:32 * m + 32, i, 128 * c:128 * (c + 1)] = \
                        xsj[:, m:512:4].astype(np.float32)
    o = np.zeros((256, F), np.float32)
    for c in range(2):
        for oc in range(2):
            acc = np.zeros((P, 512), np.float32)
            for ft in range(16):
                acc += (ytv[:, ft, c * P:(c + 1) * P]
                        .astype(np.float16).astype(np.float32).T
                        @ wtp[:, ft, oc * 512:(oc + 1) * 512])
            o[c * P:(c + 1) * P, oc * 512:(oc + 1) * 512] = acc + bvv[oc * 512:(oc + 1) * 512]
    return o


def kernel(q, k, v, W_out, b_out, apply_mask, _mock=False):
    q = np.asarray(q, np.float32)
    k = np.asarray(k, np.float32)
    v = np.asarray(v, np.float32)
    W_out = np.asarray(W_out, np.float32)
    b_out = np.asarray(b_out, np.float32)
    causal = bool(int(np.asarray(apply_mask)))
    in_maps = _shard_inputs(q, k, v, W_out, b_out)
    if _mock:
        outs = [_numpy_core(m, causal) for m in in_maps]
        return _unshard(outs)
    from concourse.bass_utils import run_bass_kernel_spmd
    nc = _get_nc(causal)
    res = run_bass_kernel_spmd(nc, in_maps, core_ids=list(range(NCORES)))
    return _unshard([r["o"] for r in res.results])
